# revision 1
# baseline (speedup 1.0000x reference)
"""Multi-head attention (dense transformer block) on 8 TRN2 NeuronCores.

Sharding: 8 cores = 4 batches x 2 head-halves.
  core c: batch b = c // 2, head half H = c % 2 (heads H*8 .. H*8+8).
  Each core computes attention for its 8 heads of its batch plus the
  partial final projection (row-shard of Wo); the host sums core pairs.
  Output bias is folded into the even core of each pair.

Per-core kernel (matmuls in float32r: full PE rate at moving dim >= 256;
walrus requires fp32r matmul operands to be produced by an instruction
whose output dtype is float32r, so all matmul-feeding tiles are f32r and
the PSUM-evac copies / DMAs do the rounding):
  0. Load pre-transposed x^T (host supplies it) into SBUF [e, s] layout.
  1. Per 2-head group g: QT_g/KT_g [128, 2048] in [d, s] layout
     (weights streamed from DRAM); V natural [s, d] for 4 heads at a
     time, with a ones column per head block ([v(64) | 1]) so the
     attn@V matmul also produces softmax row sums at psum row 64.
  2. Per head h, q-chunk (1024): scoresT [k,q] psum [128,1024] -> ACT
     exp (scale=1/8; no max subtraction: |score/8| <~ 6 for N(0,1)
     inputs) -> attn@V accumulates psum [65,1024] over 16 k chunks.
     Normalize off the critical path: DVE copy psum->SBUF, reciprocal
     of row 64, DMA-hop that row to partition 0 (gpsimd
     partition_broadcast reads physical partition 0 of the tile),
     partition_broadcast, tensor_mul.  Even local heads multiply
     straight into outT rows 0..63; odd heads go via staging + an
     SBUF->SBUF DMA into rows 64..127 (PE psum base must be 0/32/64
     and DVE has no cross-lane path).
  3. final: out[s,e] = sum_hd outT[hd,s]^T @ Wo[hd,e] (+ bias via a
     K=1 ones-row matmul) -> DRAM.  Group 3 runs q-half-major so the
     q<1024 half of the final projection can weave into its bubbles.
"""

import numpy as np

EMBED = 1024
HEADS = 16
HEAD_DIM = 64
SEQ = 2048
BATCH = 4
N_CORES = 8

LOCAL_HEADS = 8
N_GROUPS = 4
WCOLS = LOCAL_HEADS * HEAD_DIM  # 512

P = 128
NS = SEQ // P    # 16
NE = EMBED // P  # 8
VB = HEAD_DIM + 1  # 65

_cache = {}


def _emit(nc, tc, tile, mybir, make_identity, d):
    f32 = mybir.dt.float32
    f32r = mybir.dt.float32r
    EXP = mybir.ActivationFunctionType.Exp
    ctx_pools = {}

    with (
        tc.tile_pool(name="const", bufs=1) as const_pool,
        tc.tile_pool(name="xt", bufs=1) as xt_pool,
        tc.tile_pool(name="v", bufs=1) as v_pool,
        tc.tile_pool(name="qk", bufs=2) as qk_pool,
        tc.tile_pool(name="wst", bufs=1) as wst_pool,
        tc.tile_pool(name="ps_s", bufs=2, space="PSUM") as ps_s,
        tc.tile_pool(name="ps_p", bufs=2, space="PSUM") as ps_p,
        tc.tile_pool(name="ps_a", bufs=1, space="PSUM") as ps_a,
    ):
        def load_wv(half):
            wvt = wst_pool.tile([P, NE * 256], f32r, tag="wv", name="wvt")
            wv_v = d["wv"][:].rearrange("(e p) c -> p e c", e=NE, p=P)
            nc.sync.dma_start(
                out=wvt[:].rearrange("p (e c) -> p e c", e=NE, c=256),
                in_=wv_v[:, :, half * 256:(half + 1) * 256],
            )
            return wvt

        def load_wqk(name, g):
            wt = wst_pool.tile([P, NE * P], f32r, tag="wqk", bufs=2, name="wqk")
            w_v = d[name][:].rearrange("(e p) c -> p e c", e=NE, p=P)
            nc.sync.dma_start(
                out=wt[:].rearrange("p (e c) -> p e c", e=NE, c=P),
                in_=w_v[:, :, g * P:(g + 1) * P],
            )
            return [wt[:, ei * P:(ei + 1) * P] for ei in range(NE)]

        # Weights for group 0 load BEFORE the xT stream so the first
        # projection matmuls are gated only by ~2MB of xT.
        wvt_pre = load_wv(0)
        wqk_pre = {"wq": load_wqk("wq", 0), "wk": load_wqk("wk", 0)}

        # ---- phase 0: load pre-transposed x (host supplies x^T) --------
        # s-major DMA order so group-0 projections can start after the
        # first 2MB instead of the full 8MB.
        xt_big = xt_pool.tile([P, NE * SEQ], f32r, tag="xt", name="xt_big")
        for sj in range(4):
            for ei in range(NE):
                nc.sync.dma_start(
                    out=xt_big[:, ei * SEQ + sj * 512: ei * SEQ + (sj + 1) * 512],
                    in_=d["xt"][ei * P:(ei + 1) * P, sj * 512:(sj + 1) * 512],
                )

        def xt_blk(ei, s0, slen):
            return xt_big[:, ei * SEQ + s0: ei * SEQ + s0 + slen]

        # V': [128, NS*8*VB]; s-chunk si at si*8*VB, head h block at h*VB.
        vp = v_pool.tile([P, NS * LOCAL_HEADS * VB], f32r, tag="vp", name="vp")
        vp_v = vp[:].rearrange(
            "p (s h b) -> p s h b", s=NS, h=LOCAL_HEADS, b=VB
        )
        ones128 = const_pool.tile([P, P], f32, tag="ones", name="ones128")
        nc.gpsimd.memset(ones128[:], 1.0)
        # warm the ACT exp table set (~2.7us ACT_TABLE_LOAD) during the
        # DMA-bound startup instead of at the first real softmax exp.
        warm = const_pool.tile([1, 1], f32, tag="warm", name="warm")
        nc.scalar.activation(warm[:], ones128[0:1, 0:1], EXP)
        ones_r = const_pool.tile([P, P], f32r, tag="ones_r", name="ones_r")
        nc.vector.tensor_copy(ones_r[:], ones128[:])
        bo_sb = const_pool.tile([1, EMBED], f32r, tag="bo", name="bo_sb")
        nc.sync.dma_start(out=bo_sb[:], in_=d["bo"][:])
        nc.vector.tensor_copy(
            vp_v[:, :, :, HEAD_DIM:HEAD_DIM + 1],
            ones128[:].rearrange("p (a b c) -> p a b c", a=NS, b=LOCAL_HEADS, c=1),
        )

        def vp_blk(si, h):
            base = (si * LOCAL_HEADS + h) * VB
            return vp[:, base: base + VB]

        with (
            tc.tile_pool(name="exp", bufs=4) as exp_pool,
            tc.tile_pool(name="small", bufs=1) as small_pool,
            tc.tile_pool(name="outt", bufs=1) as outt_pool,
        ):
            outt_tiles = [
                outt_pool.tile([P, SEQ], f32r, tag=f"outt{g}", name=f"outt{g}")
                for g in range(N_GROUPS)
            ]

            # wo_a reuses the wv slot (free after the last V' build);
            # wo_b reuses a qt slot (free after group 2's attention) so
            # both loads overlap group 3's attention.  Declared lazily in
            # program order right before group 3 (see below).
            wo_tiles = []

            def load_wo():
                wo_a = wst_pool.tile([P, SEQ], f32r, tag="wv", name="wo_a")
                wo_b = qk_pool.tile([P, SEQ], f32r, tag="wqt", name="wo_b")
                wo_tiles.extend([wo_a, wo_b])
                for j in range(2):
                    for jj in range(2):
                        c = 2 * j + jj
                        nc.sync.dma_start(
                            out=wo_tiles[j][:, jj * 1024:(jj + 1) * 1024],
                            in_=d["wo"][c * P:(c + 1) * P, :],
                        )

            def final_proj(si_range):
                for si in si_range:
                    ot = exp_pool.tile([P, 1024], f32, tag="e", name="et")
                    for ej in range(2):
                        pool = ps_p if (si + ej) % 2 == 0 else ps_s
                        shape = [P, 512] if pool is ps_p else [P, 1024]
                        tg = "p" if pool is ps_p else "s"
                        pt = pool.tile(shape, f32, tag=tg, name="pt")
                        for c in range(4):
                            nc.tensor.matmul(
                                pt[:, 0:512],
                                outt_tiles[c][:, si * P:(si + 1) * P],
                                wo_tiles[c // 2][:, (c % 2) * 1024 + ej * 512:
                                                 (c % 2) * 1024 + (ej + 1) * 512],
                                start=(c == 0),
                                stop=False,
                                skip_group_check=True,
                            )
                        # bias via a K=1 ones-row matmul: out += 1 (x) bo
                        nc.tensor.matmul(
                            pt[:, 0:512],
                            ones_r[0:1, 0:P],
                            bo_sb[0:1, ej * 512:(ej + 1) * 512],
                            start=False,
                            stop=True,
                            skip_group_check=True,
                        )
                        nc.vector.tensor_copy(
                            ot[:, ej * 512:(ej + 1) * 512], pt[:, 0:512]
                        )
                    nc.sync.dma_start(
                        out=d["out"][si * P:(si + 1) * P, :], in_=ot[:]
                    )

            for g in range(N_GROUPS):
                # ---- V' for 4 heads (once per 2 groups) ----------------
                if g % 2 == 0:
                    half = g // 2
                    h0 = 4 * half
                    wvt = wvt_pre if half == 0 else load_wv(half)
                    for si in range(NS):
                        pt = ps_p.tile([P, 512], f32, tag="p", name="pt")
                        for ei in range(NE):
                            nc.tensor.matmul(
                                pt[:, 0:256],
                                xt_blk(ei, si * P, P),
                                wvt[:, ei * 256:(ei + 1) * 256],
                                start=(ei == 0),
                                stop=(ei == NE - 1),
                            )
                        nc.vector.tensor_copy(
                            vp_v[:, si, h0:h0 + 4, 0:HEAD_DIM],
                            pt[:, 0:256].rearrange(
                                "p (h b) -> p h b", h=4, b=HEAD_DIM
                            ),
                        )

                # ---- QT_g / KT_g ---------------------------------------
                qkt = {}
                for name in ("wq", "wk"):
                    dst = qk_pool.tile([P, SEQ], f32r, tag=f"{name}t", name=f"{name}t")
                    qkt[name] = dst
                    wtiles = wqk_pre[name] if g == 0 else load_wqk(name, g)
                    for sj in range(4):
                        pt = ps_p.tile([P, 512], f32, tag="p", name="pt")
                        for ei in range(NE):
                            nc.tensor.matmul(
                                pt[:, 0:512],
                                wtiles[ei],
                                xt_blk(ei, sj * 512, 512),
                                start=(ei == 0),
                                stop=(ei == NE - 1),
                            )
                        nc.vector.tensor_copy(
                            dst[:, sj * 512:(sj + 1) * 512], pt[:, 0:512]
                        )

                if g == 3:
                    load_wo()
                # ---- attention for heads 2g, 2g+1 ----------------------
                # Group 3 runs q-half-major so the first half of the
                # final projection (needing only q 0..1024 of outT) can
                # start while its second q-half is still computing.
                if g < 3:
                    hq = [(s, qj) for s in range(2) for qj in range(2)]
                else:
                    hq = [(s, qj) for qj in range(2) for s in range(2)]
                for sub, qj in hq:
                    h = 2 * g + sub
                    kt, qt = qkt["wk"], qkt["wq"]
                    dr = slice(sub * HEAD_DIM, (sub + 1) * HEAD_DIM)
                    # attn@V psum rows: d rows 0..63, rowsum at 64.
                    # Even head: normalize straight into outT rows 0..63;
                    # odd head: normalize into staging then SBUF->SBUF DMA
                    # into outT rows 64..127 (PE psum base must be 0/32/64
                    # and DVE has no cross-lane path).
                    if True:
                        q0 = qj * 1024
                        aps0 = ps_a.tile([VB, 512], f32, tag="a", name="aps0", bufs=2)
                        aps1 = ps_a.tile([VB, 512], f32, tag="a", name="aps1", bufs=2)
                        ahalf = [aps0, aps1]
                        for ki in range(NS):
                            sps = ps_s.tile([P, 1024], f32, tag="s", name="pt")
                            for hf in range(2):
                                nc.tensor.matmul(
                                    sps[:, hf * 512:(hf + 1) * 512],
                                    kt[dr, ki * P:(ki + 1) * P],
                                    qt[dr, q0 + hf * 512:
                                       q0 + (hf + 1) * 512],
                                    start=True,
                                    stop=True,
                                )
                            et = exp_pool.tile([P, 1024], f32r, tag="e", name="et")
                            nc.scalar.activation(
                                et[:], sps[:], EXP, scale=1.0 / 8.0
                            )
                            for hf in range(2):
                                nc.tensor.matmul(
                                    ahalf[hf][0:VB, :],
                                    vp_blk(ki, h),
                                    et[:, hf * 512:(hf + 1) * 512],
                                    start=(ki == 0),
                                    stop=(ki == NS - 1),
                                    skip_group_check=True,
                                )
                        # evacuate the accumulator immediately (frees the
                        # single psum slot); normalize downstream from SBUF
                        # in 512-wide halves (smaller slots, off the
                        # critical path).
                        for hf in range(2):
                            sl = slice(hf * 512, (hf + 1) * 512)
                            uacc = small_pool.tile(
                                [P, 512], f32, tag="uacc", name="uacc", bufs=3
                            )
                            nc.vector.tensor_copy(uacc[0:VB, :], ahalf[hf][0:VB, :])
                            rec = small_pool.tile(
                                [P, 512], f32, tag="rec", name="rec", bufs=2
                            )
                            nc.vector.reciprocal(
                                rec[HEAD_DIM:VB, :], uacc[HEAD_DIM:VB, :]
                            )
                            # partition_broadcast reads physical partition 0
                            # of the tile, so hop the row down via DMA first.
                            rec0 = small_pool.tile(
                                [P, 512], f32, tag="rec", name="rec0", bufs=2
                            )
                            nc.sync.dma_start(
                                out=rec0[0:1, :], in_=rec[HEAD_DIM:VB, :]
                            )
                            bc = small_pool.tile(
                                [P, 512], f32, tag="rec", name="bc", bufs=2
                            )
                            nc.gpsimd.partition_broadcast(
                                bc[0:HEAD_DIM, :], rec0[0:1, :]
                            )
                            if sub == 0:
                                nc.vector.tensor_mul(
                                    outt_tiles[g][0:HEAD_DIM, q0 + hf * 512:
                                                  q0 + (hf + 1) * 512],
                                    uacc[0:HEAD_DIM, :],
                                    bc[0:HEAD_DIM, :],
                                )
                            else:
                                stg = small_pool.tile(
                                    [P, 512], f32r, tag="uacc", name="stg",
                                    bufs=3
                                )
                                nc.vector.tensor_mul(
                                    stg[0:HEAD_DIM, :],
                                    uacc[0:HEAD_DIM, :],
                                    bc[0:HEAD_DIM, :],
                                )
                                nc.sync.dma_start(
                                    out=outt_tiles[g][HEAD_DIM:P,
                                                      q0 + hf * 512:
                                                      q0 + (hf + 1) * 512],
                                    in_=stg[0:HEAD_DIM, :],
                                )


            final_proj(range(0, NS))


def _build_nc():
    import concourse.mybir as mybir
    import concourse.tile as tile
    from concourse import bacc
    from concourse.masks import make_identity

    f32 = mybir.dt.float32
    f32r = mybir.dt.float32r
    nc = bacc.Bacc(
        "TRN2", target_bir_lowering=False, debug=False, num_devices=N_CORES
    )
    d = {
        "xt": nc.dram_tensor("xt", [EMBED, SEQ], f32r, kind="ExternalInput"),
        "wq": nc.dram_tensor("wq", [EMBED, WCOLS], f32r, kind="ExternalInput"),
        "wk": nc.dram_tensor("wk", [EMBED, WCOLS], f32r, kind="ExternalInput"),
        "wv": nc.dram_tensor("wv", [EMBED, WCOLS], f32r, kind="ExternalInput"),
        "wo": nc.dram_tensor("wo", [WCOLS, EMBED], f32r, kind="ExternalInput"),
        "bo": nc.dram_tensor("bo", [1, EMBED], f32r, kind="ExternalInput"),
        "out": nc.dram_tensor("out", [SEQ, EMBED], f32, kind="ExternalOutput"),
    }
    with tile.TileContext(nc) as tc:
        _emit(nc, tc, tile, mybir, make_identity, d)
    nc.compile()
    return nc


def _get_nc():
    if "nc" not in _cache:
        _cache["nc"] = _build_nc()
    return _cache["nc"]


def make_in_maps(x, Wq, Wk, Wv, Wo, bo):
    x = np.asarray(x, dtype=np.float32)
    Wq = np.asarray(Wq, dtype=np.float32)
    Wk = np.asarray(Wk, dtype=np.float32)
    Wv = np.asarray(Wv, dtype=np.float32)
    Wo = np.asarray(Wo, dtype=np.float32)
    bo = np.asarray(bo, dtype=np.float32)
    xts = [np.ascontiguousarray(x[b].T) for b in range(BATCH)]
    in_maps = []
    for c in range(N_CORES):
        b, H = c // 2, c % 2
        cs = slice(H * WCOLS, (H + 1) * WCOLS)
        bo_eff = bo if H == 0 else np.zeros_like(bo)
        in_maps.append({
            "xt": xts[b],
            "wq": np.ascontiguousarray(Wq[:, cs]),
            "wk": np.ascontiguousarray(Wk[:, cs]),
            "wv": np.ascontiguousarray(Wv[:, cs]),
            "wo": np.ascontiguousarray(Wo[cs, :]),
            "bo": np.ascontiguousarray(bo_eff.reshape(1, EMBED)),
        })
    return in_maps


def _get_runner():
    """Cached jitted SPMD callable (avoids per-call retrace).

    Output donation is unnecessary: the kernel writes every element of
    its only output, so the prezeroed buffers are reused across calls.
    """
    if "runner" in _cache:
        return _cache["runner"]
    import jax
    from jax.sharding import Mesh, NamedSharding, PartitionSpec
    from jax.experimental.shard_map import shard_map
    from concourse import mybir
    from concourse.bass2jax import (
        _bass_exec_p,
        install_neuronx_cc_hook,
        partition_id_tensor,
    )

    nc = _get_nc()
    install_neuronx_cc_hook()
    pname = nc.partition_id_tensor.name if nc.partition_id_tensor else None
    in_names, out_names, out_avals, zeros = [], [], [], []
    for alloc in nc.m.functions[0].allocations:
        if not isinstance(alloc, mybir.MemoryLocationSet):
            continue
        name = alloc.memorylocations[0].name
        if alloc.kind == "ExternalInput":
            if name != pname:
                in_names.append(name)
        elif alloc.kind == "ExternalOutput":
            shape = tuple(alloc.tensor_shape)
            dtype = mybir.dt.np(alloc.dtype)
            out_names.append(name)
            out_avals.append(jax.core.ShapedArray(shape, dtype))
            zeros.append(np.zeros(shape, dtype))
    names_all = in_names + out_names + ([pname] if pname else [])

    def _body(*args):
        operands = list(args)
        if pname is not None:
            operands.append(partition_id_tensor())
        return tuple(_bass_exec_p.bind(
            *operands,
            out_avals=tuple(out_avals),
            in_names=tuple(names_all),
            out_names=tuple(out_names),
            lowering_input_output_aliases=(),
            sim_require_finite=True,
            sim_require_nnan=True,
            nc=nc,
        ))

    devices = jax.devices()[:N_CORES]
    mesh = Mesh(np.asarray(devices), ("core",))
    nio = len(in_names) + len(out_names)
    sharded = jax.jit(
        shard_map(
            _body, mesh=mesh,
            in_specs=(PartitionSpec("core"),) * nio,
            out_specs=(PartitionSpec("core"),) * len(out_names),
            check_rep=False,
        ),
        keep_unused=True,
    )
    sh = NamedSharding(mesh, PartitionSpec("core"))
    zdev = [
        jax.device_put(np.zeros((N_CORES * z.shape[0], *z.shape[1:]), z.dtype), sh)
        for z in zeros
    ]
    _cache["runner"] = (sharded, in_names, out_names, out_avals, zdev, sh)
    return _cache["runner"]


def kernel(x, Wq, Wk, Wv, Wo, bo, trace=False):
    in_maps = make_in_maps(x, Wq, Wk, Wv, Wo, bo)
    try:
        import jax

        sharded, in_names, out_names, out_avals, zdev, sh = _get_runner()
        concat = [
            jax.device_put(
                np.concatenate([m[n] for m in in_maps], axis=0), sh
            )
            for n in in_names
        ]
        outs = sharded(*concat, *zdev)
        arr = np.asarray(outs[out_names.index("out")]).reshape(
            N_CORES, SEQ, EMBED
        )
        out = np.empty((BATCH, SEQ, EMBED), dtype=np.float32)
        for b in range(BATCH):
            out[b] = arr[2 * b] + arr[2 * b + 1]
        return out
    except Exception:
        from concourse.bass_utils import run_bass_kernel_spmd

        nc = _get_nc()
        res = run_bass_kernel_spmd(
            nc, in_maps, list(range(N_CORES)), trace=trace
        )
        _cache["last_result"] = res
        out = np.empty((BATCH, SEQ, EMBED), dtype=np.float32)
        for b in range(BATCH):
            out[b] = res.results[2 * b]["out"] + res.results[2 * b + 1]["out"]
        return out



# revision 5
# speedup vs baseline: 3.0555x; 3.0555x over previous
"""Multi-head attention (dense transformer block) on 8 TRN2 NeuronCores.

Sharding: 8 cores = 4 batches x 2 head-halves.
  core c: batch b = c // 2, head half H = c % 2 (heads H*8 .. H*8+8).
  Each core computes attention for its 8 heads of its batch plus the
  partial final projection (row-shard of Wo); the host sums core pairs.
  Output bias is folded into the even core of each pair.

Per-core kernel. All SBUF-resident operands are bf16 (halves the input
DMA and enables FWL weight loads); PSUM accumulation stays fp32, and
the attention probabilities/V go through fp8e4 with a DoubleRow matmul:

  0. Load pre-transposed x^T (host supplies bf16 x^T) into SBUF [e, s].
  1. Per 2-head group g: QT_g/KT_g [128, 2048] in [d, s] layout; V for
     4 heads at a time in fp8e4 with a DoubleRow-interleaved layout
     [p][t=si//2][h][plane=si%2][80] and a ones column per head block
     so the attn@V matmul also produces softmax row sums at psum row 64.
  2. Attention runs per head PAIR (2g, 2g+1) and q-chunk of 512.  The
     two heads' scoresT matmuls are K=64 with lhsT/rhs at base
     partitions 0 and 64 -> distinct PE row-groups -> the two matmuls
     stream CONCURRENTLY through the array (row tiling), restoring full
     PE utilization for the d=64 contraction.  Both write one pair psum
     tile [128, (headA 512 | headB 512)], which a single ACT exp
     converts to fp8e4 probabilities et (scale=1/8, bias=-2 keeps
     exp <= e^4 ~ 55 < 240 = fp8e4 max; the bias cancels in the softmax
     normalization).  Per ki-pair t, attn@V is one fp8 DoubleRow matmul
     per head (contraction 256 = 2 planes x 128 partitions, ~1.44x PE).
     Normalization off the critical path as before: evac accum, DVE
     reciprocal of row 64, DMA-hop to partition 0, gpsimd
     partition_broadcast, tensor_mul.  Even local heads multiply
     straight into outT rows 0..63; odd heads go via staging + an
     SBUF->SBUF DMA into rows 64..127 (PE psum base must be 0/32/64
     and DVE has no cross-lane path).
  3. final: out[s,e] = sum_hd outT[hd,s]^T @ Wo[hd,e] (+ bias via a
     K=1 ones-row matmul) -> DRAM.  q-chunks ascend, so the q<1024
     half of the final projection can weave into group 3's bubbles.
"""

import numpy as np

EMBED = 1024
HEADS = 16
HEAD_DIM = 64
SEQ = 2048
BATCH = 4
N_CORES = 8

LOCAL_HEADS = 8
N_GROUPS = 4
WCOLS = LOCAL_HEADS * HEAD_DIM  # 512

P = 128
NS = SEQ // P    # 16
NE = EMBED // P  # 8
VB = HEAD_DIM + 1  # 65
VSTRIDE = 80     # fp8 V plane stride (65 padded; DoubleRow needs %16==0)
NT = NS // 2     # 8 ki pairs
QC = 512         # q-chunk
NQ = SEQ // QC   # 4

# fp8 attn@V (DoubleRow) measured 2.4e-2 rel err vs the 2e-2 gate: the
# attention output is a weighted mean whose magnitude shrinks by the same
# sqrt(N_eff) factor as the fp8 quantization error-of-mean, so fp8's ~3.6%
# RMS per-element error passes through undiminished (~1.7e-2 from et and
# from V each).  bf16 keeps the total at ~4.4e-3.
FP8_ATTNV = False
EXP_BIAS = -2.0  # keeps exp output within fp8e4 range; cancels in softmax

TIMING_REPEATS = 4

_cache = {}


def _emit(nc, tc, tile, mybir, d):
    f32 = mybir.dt.float32
    bf16 = mybir.dt.bfloat16
    fp8 = mybir.dt.float8e4
    EXP = mybir.ActivationFunctionType.Exp
    DR = mybir.MatmulPerfMode.DoubleRow

    with (
        tc.tile_pool(name="const", bufs=1) as const_pool,
        tc.tile_pool(name="xt", bufs=1) as xt_pool,
        tc.tile_pool(name="v", bufs=1) as v_pool,
        tc.tile_pool(name="qk", bufs=2) as qk_pool,
        tc.tile_pool(name="wst", bufs=1) as wst_pool,
        tc.tile_pool(name="ps_s", bufs=2, space="PSUM") as ps_s,
        tc.tile_pool(name="ps_p", bufs=2, space="PSUM") as ps_p,
        tc.tile_pool(name="ps_a", bufs=2, space="PSUM") as ps_a,
    ):
        def load_wv(half):
            wvt = wst_pool.tile([P, NE * 256], bf16, tag="wv", name="wvt")
            wv_v = d["wv"][:].rearrange("(e p) c -> p e c", e=NE, p=P)
            nc.sync.dma_start(
                out=wvt[:].rearrange("p (e c) -> p e c", e=NE, c=256),
                in_=wv_v[:, :, half * 256:(half + 1) * 256],
            )
            return wvt

        def load_wqk(name, g):
            wt = wst_pool.tile([P, NE * P], bf16, tag="wqk", bufs=2, name="wqk")
            w_v = d[name][:].rearrange("(e p) c -> p e c", e=NE, p=P)
            nc.sync.dma_start(
                out=wt[:].rearrange("p (e c) -> p e c", e=NE, c=P),
                in_=w_v[:, :, g * P:(g + 1) * P],
            )
            return [wt[:, ei * P:(ei + 1) * P] for ei in range(NE)]

        # Weights for group 0 load BEFORE the xT stream so the first
        # projection matmuls are gated only by ~1MB of xT.
        wvt_pre = load_wv(0)
        wqk_pre = {"wq": load_wqk("wq", 0), "wk": load_wqk("wk", 0)}

        # ---- phase 0: load pre-transposed x (host supplies bf16 x^T) ---
        xt_big = xt_pool.tile([P, NE * SEQ], bf16, tag="xt", name="xt_big")
        for sj in range(4):
            for ei in range(NE):
                nc.sync.dma_start(
                    out=xt_big[:, ei * SEQ + sj * 512: ei * SEQ + (sj + 1) * 512],
                    in_=d["xt"][ei * P:(ei + 1) * P, sj * 512:(sj + 1) * 512],
                )

        def xt_blk(ei, s0, slen):
            return xt_big[:, ei * SEQ + s0: ei * SEQ + s0 + slen]

        if FP8_ATTNV:
            # V': [p][t=si//2][h][plane=si%2][VSTRIDE] fp8, ones at col 64
            vp = v_pool.tile(
                [P, NT * LOCAL_HEADS * 2 * VSTRIDE], fp8, tag="vp", name="vp"
            )
            vp_r = vp[:].rearrange(
                "p (t h pl c) -> p t h pl c",
                t=NT, h=LOCAL_HEADS, pl=2, c=VSTRIDE,
            )
            ones_blocks = NT * LOCAL_HEADS * 2
            ones_view = vp[:].rearrange(
                "p (blk c) -> p blk c", blk=ones_blocks, c=VSTRIDE
            )[:, :, HEAD_DIM:HEAD_DIM + 1]
        else:
            vp = v_pool.tile(
                [P, NS * LOCAL_HEADS * VB], bf16, tag="vp", name="vp"
            )
            vp_r = vp[:].rearrange(
                "p (s h b) -> p s h b", s=NS, h=LOCAL_HEADS, b=VB
            )
            ones_blocks = NS * LOCAL_HEADS
            ones_view = vp[:].rearrange(
                "p (blk c) -> p blk c", blk=ones_blocks, c=VB
            )[:, :, HEAD_DIM:HEAD_DIM + 1]

        ones128 = const_pool.tile([P, P], f32, tag="ones", name="ones128")
        nc.gpsimd.memset(ones128[:], 1.0)
        ebias = const_pool.tile([P, 1], f32, tag="ebias", name="ebias")
        nc.gpsimd.memset(ebias[:], EXP_BIAS)
        # warm the ACT exp table set (~2.7us ACT_TABLE_LOAD) during the
        # DMA-bound startup instead of at the first real softmax exp.
        warm = const_pool.tile([1, 1], f32, tag="warm", name="warm")
        nc.scalar.activation(warm[:], ones128[0:1, 0:1], EXP)
        ones_bf = const_pool.tile([1, P], bf16, tag="ones_bf", name="ones_bf")
        nc.vector.tensor_copy(ones_bf[:], ones128[0:1, :])
        bo_sb = const_pool.tile([1, EMBED], bf16, tag="bo", name="bo_sb")
        nc.sync.dma_start(out=bo_sb[:], in_=d["bo"][:])
        nc.vector.tensor_copy(
            ones_view,
            ones128[:].rearrange("p (a b) -> p a b", a=P, b=1)[
                :, 0:ones_blocks, :
            ],
        )

        with (
            tc.tile_pool(name="exp", bufs=1) as exp_pool,
            tc.tile_pool(name="small", bufs=1) as small_pool,
            tc.tile_pool(name="outt", bufs=1) as outt_pool,
        ):
            outt_tiles = [
                outt_pool.tile([P, SEQ], bf16, tag=f"outt{g}", name=f"outt{g}")
                for g in range(N_GROUPS)
            ]

            # wo_a reuses the wv slot (free after the last V' build);
            # wo_b reuses a qt slot (free after group 2's attention) so
            # both loads overlap group 3's attention.
            wo_tiles = []

            def load_wo():
                wo_a = wst_pool.tile([P, SEQ], bf16, tag="wv", name="wo_a")
                wo_b = qk_pool.tile([P, SEQ], bf16, tag="wqt", name="wo_b")
                wo_tiles.extend([wo_a, wo_b])
                for j in range(2):
                    for jj in range(2):
                        c = 2 * j + jj
                        nc.sync.dma_start(
                            out=wo_tiles[j][:, jj * 1024:(jj + 1) * 1024],
                            in_=d["wo"][c * P:(c + 1) * P, :],
                        )

            def final_proj(si_range):
                for si in si_range:
                    ot = exp_pool.tile(
                        [P, 1024], f32, tag="ot", name="ot", bufs=2
                    )
                    for ej in range(2):
                        pool = ps_p if (si + ej) % 2 == 0 else ps_s
                        shape = [P, 512] if pool is ps_p else [P, 1024]
                        tg = "p" if pool is ps_p else "s"
                        pt = pool.tile(shape, f32, tag=tg, name="pt")
                        for c in range(4):
                            nc.tensor.matmul(
                                pt[:, 0:512],
                                outt_tiles[c][:, si * P:(si + 1) * P],
                                wo_tiles[c // 2][:, (c % 2) * 1024 + ej * 512:
                                                 (c % 2) * 1024 + (ej + 1) * 512],
                                start=(c == 0),
                                stop=False,
                                skip_group_check=True,
                            )
                        # bias via a K=1 ones-row matmul: out += 1 (x) bo
                        nc.tensor.matmul(
                            pt[:, 0:512],
                            ones_bf[0:1, 0:P],
                            bo_sb[0:1, ej * 512:(ej + 1) * 512],
                            start=False,
                            stop=True,
                            skip_group_check=True,
                        )
                        nc.vector.tensor_copy(
                            ot[:, ej * 512:(ej + 1) * 512], pt[:, 0:512]
                        )
                    nc.sync.dma_start(
                        out=d["out"][si * P:(si + 1) * P, :], in_=ot[:]
                    )

            def normalize(aps, g, qc, sub):
                q0 = qc * QC
                uacc = small_pool.tile(
                    [P, QC], f32, tag="uacc", name="uacc", bufs=3
                )
                nc.vector.tensor_copy(uacc[0:VB, :], aps[0:VB, :])
                rec = small_pool.tile(
                    [P, QC], f32, tag="rec", name="rec", bufs=2
                )
                nc.vector.reciprocal(
                    rec[HEAD_DIM:VB, :], uacc[HEAD_DIM:VB, :]
                )
                # partition_broadcast reads physical partition 0 of the
                # tile, so hop the row down via DMA first.
                rec0 = small_pool.tile(
                    [P, QC], f32, tag="rec", name="rec0", bufs=2
                )
                nc.sync.dma_start(
                    out=rec0[0:1, :], in_=rec[HEAD_DIM:VB, :]
                )
                bc = small_pool.tile(
                    [P, QC], f32, tag="rec", name="bc", bufs=2
                )
                nc.gpsimd.partition_broadcast(
                    bc[0:HEAD_DIM, :], rec0[0:1, :]
                )
                if sub == 0:
                    nc.vector.tensor_mul(
                        outt_tiles[g][0:HEAD_DIM, q0:q0 + QC],
                        uacc[0:HEAD_DIM, :],
                        bc[0:HEAD_DIM, :],
                    )
                else:
                    stg = small_pool.tile(
                        [P, QC], bf16, tag="stg", name="stg", bufs=2
                    )
                    nc.vector.tensor_mul(
                        stg[0:HEAD_DIM, :],
                        uacc[0:HEAD_DIM, :],
                        bc[0:HEAD_DIM, :],
                    )
                    nc.sync.dma_start(
                        out=outt_tiles[g][HEAD_DIM:P, q0:q0 + QC],
                        in_=stg[0:HEAD_DIM, :],
                    )

            for g in range(N_GROUPS):
                # ---- V' for 4 heads (once per 2 groups) ----------------
                if g % 2 == 0:
                    half = g // 2
                    h0 = 4 * half
                    wvt = wvt_pre if half == 0 else load_wv(half)
                    for si in range(NS):
                        pt = ps_p.tile([P, 512], f32, tag="p", name="pt")
                        for ei in range(NE):
                            nc.tensor.matmul(
                                pt[:, 0:256],
                                xt_blk(ei, si * P, P),
                                wvt[:, ei * 256:(ei + 1) * 256],
                                start=(ei == 0),
                                stop=(ei == NE - 1),
                            )
                        if FP8_ATTNV:
                            dst = vp_r[:, si // 2, h0:h0 + 4, si % 2,
                                       0:HEAD_DIM]
                        else:
                            dst = vp_r[:, si, h0:h0 + 4, 0:HEAD_DIM]
                        nc.vector.tensor_copy(
                            dst,
                            pt[:, 0:256].rearrange(
                                "p (h b) -> p h b", h=4, b=HEAD_DIM
                            ),
                        )

                # ---- QT_g / KT_g ---------------------------------------
                qkt = {}
                for name in ("wq", "wk"):
                    dst = qk_pool.tile(
                        [P, SEQ], bf16, tag=f"{name}t", name=f"{name}t"
                    )
                    qkt[name] = dst
                    wtiles = wqk_pre[name] if g == 0 else load_wqk(name, g)
                    for sj in range(4):
                        pt = ps_p.tile([P, 512], f32, tag="p", name="pt")
                        for ei in range(NE):
                            nc.tensor.matmul(
                                pt[:, 0:512],
                                wtiles[ei],
                                xt_blk(ei, sj * 512, 512),
                                start=(ei == 0),
                                stop=(ei == NE - 1),
                            )
                        nc.vector.tensor_copy(
                            dst[:, sj * 512:(sj + 1) * 512], pt[:, 0:512]
                        )

                if g == 3:
                    load_wo()

                # ---- attention for the head pair (2g, 2g+1) ------------
                kt, qt = qkt["wk"], qkt["wq"]
                hA, hB = 2 * g, 2 * g + 1
                for qc in range(NQ):
                    q0 = qc * QC
                    apsA = ps_a.tile([VB, QC], f32, tag="a", name="apsA")
                    apsB = ps_a.tile([VB, QC], f32, tag="a", name="apsB")
                    et3 = None
                    for ki in range(NS):
                        sps = ps_s.tile([P, 1024], f32, tag="s", name="sps")
                        ks = slice(ki * P, (ki + 1) * P)
                        qs = slice(q0, q0 + QC)
                        # two heads stream concurrently: distinct PE
                        # row-groups (base partitions 0 / 64)
                        nc.tensor.matmul(
                            sps[:, 0:512],
                            kt[0:HEAD_DIM, ks], qt[0:HEAD_DIM, qs],
                            start=True, stop=True,
                        )
                        nc.tensor.matmul(
                            sps[:, 512:1024],
                            kt[HEAD_DIM:P, ks], qt[HEAD_DIM:P, qs],
                            start=True, stop=True,
                        )
                        sps_v = sps[:].rearrange(
                            "p (h n) -> p h n", h=2, n=QC
                        )
                        if FP8_ATTNV:
                            t = ki // 2
                            pl = ki % 2
                            if pl == 0:
                                et3 = exp_pool.tile(
                                    [P, 2048], fp8, tag="et", name="et3",
                                    bufs=2,
                                )
                            et3_v = et3[:].rearrange(
                                "p (h pl n) -> p h pl n", h=2, pl=2, n=QC
                            )
                            nc.scalar.activation(
                                et3_v[:, :, pl, :], sps_v, EXP,
                                scale=1.0 / 8.0, bias=ebias[:],
                            )
                            if pl == 1:
                                for sub, aps in ((0, apsA), (1, apsB)):
                                    nc.tensor.matmul(
                                        aps[0:VB, :],
                                        vp_r[:, t, 2 * g + sub, :, 0:VB],
                                        et3_v[:, sub, :, :],
                                        start=(t == 0),
                                        stop=(t == NT - 1),
                                        perf_mode=DR,
                                        skip_group_check=True,
                                    )
                        else:
                            et = exp_pool.tile(
                                [P, 1024], bf16, tag="et", name="et", bufs=4
                            )
                            nc.scalar.activation(
                                et[:], sps[:], EXP, scale=1.0 / 8.0
                            )
                            for sub, aps in ((0, apsA), (1, apsB)):
                                nc.tensor.matmul(
                                    aps[0:VB, :],
                                    vp_r[:, ki, 2 * g + sub, 0:VB],
                                    et[:, sub * QC:(sub + 1) * QC],
                                    start=(ki == 0),
                                    stop=(ki == NS - 1),
                                    skip_group_check=True,
                                )
                    normalize(apsA, g, qc, 0)
                    normalize(apsB, g, qc, 1)

            final_proj(range(0, NS))


def _build_nc(repeats=1):
    import concourse.mybir as mybir
    import concourse.tile as tile
    from concourse import bacc

    f32 = mybir.dt.float32
    bf16 = mybir.dt.bfloat16
    nc = bacc.Bacc(
        "TRN2", target_bir_lowering=False, debug=False, num_devices=N_CORES
    )
    d = {
        "xt": nc.dram_tensor("xt", [EMBED, SEQ], bf16, kind="ExternalInput"),
        "wq": nc.dram_tensor("wq", [EMBED, WCOLS], bf16, kind="ExternalInput"),
        "wk": nc.dram_tensor("wk", [EMBED, WCOLS], bf16, kind="ExternalInput"),
        "wv": nc.dram_tensor("wv", [EMBED, WCOLS], bf16, kind="ExternalInput"),
        "wo": nc.dram_tensor("wo", [WCOLS, EMBED], bf16, kind="ExternalInput"),
        "bo": nc.dram_tensor("bo", [1, EMBED], bf16, kind="ExternalInput"),
        "out": nc.dram_tensor("out", [SEQ, EMBED], f32, kind="ExternalOutput"),
    }
    with tile.TileContext(nc) as tc:
        for _ in range(repeats):
            _emit(nc, tc, tile, mybir, d)
    nc.compile()
    return nc


def _get_nc(repeats=1):
    key = f"nc{repeats}"
    if key not in _cache:
        _cache[key] = _build_nc(repeats)
    return _cache[key]


def make_in_maps(x, Wq, Wk, Wv, Wo, bo):
    import ml_dtypes

    bf16 = ml_dtypes.bfloat16
    x = np.asarray(x, dtype=np.float32)
    Wq = np.asarray(Wq, dtype=np.float32)
    Wk = np.asarray(Wk, dtype=np.float32)
    Wv = np.asarray(Wv, dtype=np.float32)
    Wo = np.asarray(Wo, dtype=np.float32)
    bo = np.asarray(bo, dtype=np.float32)
    xts = [np.ascontiguousarray(x[b].T).astype(bf16) for b in range(BATCH)]
    in_maps = []
    for c in range(N_CORES):
        b, H = c // 2, c % 2
        cs = slice(H * WCOLS, (H + 1) * WCOLS)
        bo_eff = bo if H == 0 else np.zeros_like(bo)
        in_maps.append({
            "xt": xts[b],
            "wq": np.ascontiguousarray(Wq[:, cs]).astype(bf16),
            "wk": np.ascontiguousarray(Wk[:, cs]).astype(bf16),
            "wv": np.ascontiguousarray(Wv[:, cs]).astype(bf16),
            "wo": np.ascontiguousarray(Wo[cs, :]).astype(bf16),
            "bo": np.ascontiguousarray(bo_eff.reshape(1, EMBED)).astype(bf16),
        })
    return in_maps


def _get_runner(repeats=1):
    """Cached jitted SPMD callable (avoids per-call retrace)."""
    key = f"runner{repeats}"
    if key in _cache:
        return _cache[key]
    import jax
    from jax.sharding import Mesh, NamedSharding, PartitionSpec
    from jax.experimental.shard_map import shard_map
    from concourse import mybir
    from concourse.bass2jax import (
        _bass_exec_p,
        install_neuronx_cc_hook,
        partition_id_tensor,
    )

    nc = _get_nc(repeats)
    install_neuronx_cc_hook()
    pname = nc.partition_id_tensor.name if nc.partition_id_tensor else None
    in_names, out_names, out_avals, zeros = [], [], [], []
    for alloc in nc.m.functions[0].allocations:
        if not isinstance(alloc, mybir.MemoryLocationSet):
            continue
        name = alloc.memorylocations[0].name
        if alloc.kind == "ExternalInput":
            if name != pname:
                in_names.append(name)
        elif alloc.kind == "ExternalOutput":
            shape = tuple(alloc.tensor_shape)
            dtype = mybir.dt.np(alloc.dtype)
            out_names.append(name)
            out_avals.append(jax.core.ShapedArray(shape, dtype))
            zeros.append(np.zeros(shape, dtype))
    names_all = in_names + out_names + ([pname] if pname else [])

    def _body(*args):
        operands = list(args)
        if pname is not None:
            operands.append(partition_id_tensor())
        return tuple(_bass_exec_p.bind(
            *operands,
            out_avals=tuple(out_avals),
            in_names=tuple(names_all),
            out_names=tuple(out_names),
            lowering_input_output_aliases=(),
            sim_require_finite=True,
            sim_require_nnan=True,
            nc=nc,
        ))

    devices = jax.devices()[:N_CORES]
    mesh = Mesh(np.asarray(devices), ("core",))
    nio = len(in_names) + len(out_names)
    sharded = jax.jit(
        shard_map(
            _body, mesh=mesh,
            in_specs=(PartitionSpec("core"),) * nio,
            out_specs=(PartitionSpec("core"),) * len(out_names),
            check_rep=False,
        ),
        keep_unused=True,
    )
    sh = NamedSharding(mesh, PartitionSpec("core"))
    zdev = [
        jax.device_put(np.zeros((N_CORES * z.shape[0], *z.shape[1:]), z.dtype), sh)
        for z in zeros
    ]
    _cache[key] = (sharded, in_names, out_names, out_avals, zdev, sh)
    return _cache[key]


def kernel(x, Wq, Wk, Wv, Wo, bo, trace=False):
    in_maps = make_in_maps(x, Wq, Wk, Wv, Wo, bo)
    try:
        import jax

        sharded, in_names, out_names, out_avals, zdev, sh = _get_runner()
        concat = [
            jax.device_put(
                np.concatenate([m[n] for m in in_maps], axis=0), sh
            )
            for n in in_names
        ]
        outs = sharded(*concat, *zdev)
        arr = np.asarray(outs[out_names.index("out")]).reshape(
            N_CORES, SEQ, EMBED
        )
        out = np.empty((BATCH, SEQ, EMBED), dtype=np.float32)
        for b in range(BATCH):
            out[b] = arr[2 * b] + arr[2 * b + 1]
        return out
    except Exception:
        from concourse.bass_utils import run_bass_kernel_spmd

        nc = _get_nc()
        res = run_bass_kernel_spmd(
            nc, in_maps, list(range(N_CORES)), trace=trace
        )
        _cache["last_result"] = res
        out = np.empty((BATCH, SEQ, EMBED), dtype=np.float32)
        for b in range(BATCH):
            out[b] = res.results[2 * b]["out"] + res.results[2 * b + 1]["out"]
        return out
